# revision 1
# baseline (speedup 1.0000x reference)
"""Trainium2 Bass kernel: causal multi-head self-attention with RoPE.

Problem: B=4, S=2048, D=1024, H=16, DK=64.  out = softmax(causal(qk^T/8)) v @ wo^T
with q,k RoPE-rotated.

Sharding: 8 cores = (batch b in 0..3) x (head-group g in 0..1, 8 heads each).
Each core computes its batch's QKV for its 8 heads, causal attention, and a
partial output projection; the host sums the two group-partials per batch.

Structure (per core), pair-outer so attention (ACT-heavy) overlaps later
pairs' projections (PE-heavy):
  v projection first (all heads), then per head-pair p:
    q/k projection for p -> RoPE (split layout, swap via SBUF->SBUF DMA)
    attention for p over all q tiles: scores transposed ST[k,q] with K=64 and
    the two heads packed via tile_position row groups; kc2-batched scores into
    [128,1024] 2-bank PSUM tiles so exp runs at FD=1024; E bf16; causal mask
    on the diagonal kc2 groups; attn@V with ones-augmented V (M=65, row 64 =
    softmax denominator), accumulated over k chunks in PSUM.
    Normalization per (pair, qc): reciprocal_approx_fast + broadcast-DMA.
  Output projection at the end: out.T[m,q] = woT.T @ A.T over 4 J-chunks.
"""
import os
import sys

for _p in ("/opt/trn_rl_repo", "/root/.axon_site/_ro/trn_rl_repo"):
    if os.path.isdir(_p) and _p not in sys.path:
        sys.path.insert(0, _p)

import numpy as np
import ml_dtypes

import concourse.bass as bass
import concourse.mybir as mybir
import concourse.tile as tile
from concourse import bacc
from concourse.bass_utils import run_bass_kernel_spmd

B, S, D, H = 4, 2048, 1024, 16
DK = D // H          # 64
HG = 8               # heads per group
NG = 2               # head groups (cores per batch)
THETA = 10000.0
NCORES = 8

BF16 = mybir.dt.bfloat16
F32 = mybir.dt.float32
bf16 = ml_dtypes.bfloat16

QT = 512             # q tile width (free dim)
NQT = S // QT        # 4
NKT = S // 128       # 16 k chunks
NJT = HG * DK // 128  # 4 j-tiles (head pairs)
NDC = D // 128       # 8 d chunks
NMT = D // 128       # 8 output m tiles


def _build_nc():
    nc = bacc.Bacc("TRN2", target_bir_lowering=False, debug=False)
    xT = nc.dram_tensor("xT", [D, S], BF16, kind="ExternalInput").ap()
    wqT = nc.dram_tensor("wqT", [D, HG * DK], BF16, kind="ExternalInput").ap()
    wkT = nc.dram_tensor("wkT", [D, HG * DK], BF16, kind="ExternalInput").ap()
    wvT = nc.dram_tensor("wvT", [D, HG * DK], BF16, kind="ExternalInput").ap()
    woT = nc.dram_tensor("woT", [HG * DK, D], BF16, kind="ExternalInput").ap()
    c128 = nc.dram_tensor("c128", [128, S], BF16, kind="ExternalInput").ap()
    s128 = nc.dram_tensor("s128", [128, S], BF16, kind="ExternalInput").ap()
    maskd = nc.dram_tensor("maskd", [128, 4, QT], BF16, kind="ExternalInput").ap()
    outT = nc.dram_tensor("outT", [D, S], F32, kind="ExternalOutput").ap()

    from contextlib import ExitStack
    with tile.TileContext(nc) as tc, ExitStack() as stk:
        pp = stk.enter_context(tc.tile_pool(name="persist", bufs=1))
        ep = stk.enter_context(tc.tile_pool(name="epool", bufs=6))
        sp = stk.enter_context(tc.tile_pool(name="smalls", bufs=2))
        qw = stk.enter_context(tc.tile_pool(name="qkvwork", bufs=2))
        ps_st = stk.enter_context(
            tc.tile_pool(name="ps_st", bufs=2, space="PSUM"))
        ps_ov = stk.enter_context(
            tc.tile_pool(name="ps_ov", bufs=2, space="PSUM"))
        ps_qkv = stk.enter_context(
            tc.tile_pool(name="ps_qkv", bufs=2, space="PSUM"))

        # ---------------- persistent tiles ----------------
        wo_sb = pp.tile([128, NJT, D], BF16)
        m_sb = pp.tile([128, 4, QT], BF16)
        qrot = pp.tile([128, NJT, S], BF16)
        krot = pp.tile([128, NJT, S], BF16)
        v_aug = pp.tile([128, NKT, HG, 66], BF16)
        a_t = pp.tile([128, NJT, S], BF16)
        xT_sb = pp.tile([128, NDC, S], BF16)
        wq_sb = pp.tile([128, NDC, HG * DK], BF16)
        wk_sb = pp.tile([128, NDC, HG * DK], BF16)
        c_sb = pp.tile([128, S], BF16)
        s_sb = pp.tile([128, S], BF16)

        nc.gpsimd.memset(v_aug[:, :, :, 64:65], 1.0)

        # ---------------- v projection (all heads) ----------------
        with tc.tile_pool(name="wvtmp", bufs=1) as wvp:
            wv_sb = wvp.tile([128, NDC, HG * DK], BF16)
            for dc in range(NDC):
                nc.sync.dma_start(wv_sb[:, dc, :],
                                  wvT[dc * 128:(dc + 1) * 128, :])
            for dc in range(NDC):
                nc.sync.dma_start(xT_sb[:, dc, :],
                                  xT[dc * 128:(dc + 1) * 128, :])
            for dc in range(NDC):
                nc.sync.dma_start(wq_sb[:, dc, :],
                                  wqT[dc * 128:(dc + 1) * 128, :])
                nc.sync.dma_start(wk_sb[:, dc, :],
                                  wkT[dc * 128:(dc + 1) * 128, :])
            for jc in range(NJT):
                nc.sync.dma_start(wo_sb[:, jc, :],
                                  woT[jc * 128:(jc + 1) * 128, :])
            nc.sync.dma_start(c_sb[:], c128[:])
            nc.sync.dma_start(s_sb[:], s128[:])
            nc.sync.dma_start(m_sb[:], maskd[:])
            for tt in range(NKT):
                ps = ps_qkv.tile([128, QT], F32, tag="qv")
                for dc in range(NDC):
                    nc.tensor.matmul(
                        ps[:],
                        xT_sb[:, dc, tt * 128:(tt + 1) * 128],
                        wv_sb[:, dc, :],
                        start=(dc == 0), stop=(dc == NDC - 1))
                nc.vector.tensor_copy(
                    v_aug[:, tt, :, 0:64],
                    ps[:].rearrange("p (h d) -> p h d", h=HG))

        # ------------- projections + dual-pair interleaved attention ---------
        # Two phases of two pairs each: projections (PE-heavy), then the two
        # pairs' attention interleaved group-by-group so the scheduler always
        # has an independent stream to hide exp latency with.
        def proj_unit(pair, name, w_sb, pre, tn):
            ps = ps_qkv.tile([128, QT], F32, tag="qv",
                             name=f"ps{name}{pair}{tn}")
            for dc in range(NDC):
                nc.tensor.matmul(
                    ps[:],
                    w_sb[:, dc, pair * 128:(pair + 1) * 128],
                    xT_sb[:, dc, tn * QT:(tn + 1) * QT],
                    start=(dc == 0), stop=(dc == NDC - 1))
            nc.vector.tensor_copy(pre[:, tn * QT:(tn + 1) * QT], ps[:])

        def rope_unit(pair, name, pre, dst):
            swp = qw.tile([128, S], BF16, tag="swp", name=f"swp{name}{pair}")
            for a in range(4):
                lo, sw = 32 * a, 32 * (a ^ 1)
                nc.sync.dma_start(swp[lo:lo + 32, :], pre[sw:sw + 32, :])
            nc.vector.tensor_mul(dst[:, pair, :], pre[:], c_sb[:])
            nc.vector.tensor_mul(swp[:], swp[:], s_sb[:])
            nc.vector.tensor_add(dst[:, pair, :], dst[:, pair, :], swp[:])

        def emit_scores(pair, qc, g):
            st0 = ps_st.tile([128, 2 * QT], F32, tag="st",
                             name=f"st0_{pair}{qc}{g}")
            st1 = ps_st.tile([128, 2 * QT], F32, tag="st",
                             name=f"st1_{pair}{qc}{g}")
            for half in range(2):
                kc = 2 * g + half
                for h01, st in ((0, st0), (1, st1)):
                    lo = 64 * h01
                    nc.tensor.matmul(
                        st[:, half * QT:(half + 1) * QT],
                        krot[lo:lo + 64, pair, kc * 128:(kc + 1) * 128],
                        qrot[lo:lo + 64, pair, qc * QT:(qc + 1) * QT],
                        start=True, stop=True,
                        tile_position=(lo, 0))
            return st0, st1

        def emit_tail(pair, qc, g, st0, st1, ov0, ov1, last):
            ngrp = 2 * qc + 2
            for h01, st, ov in ((0, st0, ov0), (1, st1, ov1)):
                e = ep.tile([128, 2 * QT], BF16, tag="e",
                            name=f"e{pair}{qc}{g}{h01}")
                nc.scalar.activation(
                    e[:], st[:], mybir.ActivationFunctionType.Exp,
                    scale=0.125)
                if g >= 2 * qc:    # diagonal band
                    par = g - 2 * qc
                    e3 = e[:].rearrange("p (a q) -> p a q", a=2)
                    nc.vector.tensor_mul(
                        e3, e3, m_sb[:, 2 * par:2 * par + 2, :])
                for half in range(2):
                    kc = 2 * g + half
                    nc.tensor.matmul(
                        ov[:],
                        v_aug[:, kc, 2 * pair + h01, 0:65],
                        e[:, half * QT:(half + 1) * QT],
                        start=(kc == 0),
                        stop=(last and half == 1))

        def emit_evac(pair, qc, ov0, ov1):
            """Part A: free the ov PSUM banks and stage the denominators."""
            den = sp.tile([2, QT], F32, tag="den", bufs=3,
                          name=f"den{pair}{qc}")
            ous = []
            for h01, ov in ((0, ov0), (1, ov1)):
                ou = ep.tile([65, QT], BF16, tag="ou", bufs=6,
                             name=f"ou{pair}{qc}{h01}")
                nc.vector.tensor_copy(ou[:], ov[:])
                nc.gpsimd.dma_start(den[h01:h01 + 1, :], ou[64:65, :])
                ous.append(ou)
            return den, ous

        def emit_recip(pair, qc, den):
            """Part B1: reciprocal + broadcast DMAs (deferred one qc)."""
            recip = sp.tile([2, QT], F32, tag="recip", name=f"rcp{pair}{qc}")
            nc.vector.reciprocal_approx_fast(recip[:], den[:])
            rbs = []
            for h01 in range(2):
                rb = sp.tile([64, QT], BF16, tag="rb", bufs=5,
                             name=f"rb{pair}{qc}{h01}")
                nc.gpsimd.dma_start(
                    rb[:],
                    recip[h01:h01 + 1, :]
                    .unsqueeze(1).to_broadcast((1, 64, QT)))
                rbs.append(rb)
            return rbs

        def emit_div(pair, qc, ous, rbs):
            """Part B2: the normalize multiplies (deferred further)."""
            nc.vector.tensor_mul(
                a_t[0:64, pair, qc * QT:(qc + 1) * QT],
                ous[0][0:64, :], rbs[0][:])
            an = sp.tile([64, QT], BF16, tag="an", bufs=3,
                         name=f"an{pair}{qc}")
            nc.vector.tensor_mul(an[:], ous[1][0:64, :], rbs[1][:])
            nc.sync.dma_start(
                a_t[64:128, pair, qc * QT:(qc + 1) * QT], an[:])

        def outproj_unit(qc, mt):
            op = ps_qkv.tile([128, QT], F32, tag="qv", name=f"op{qc}{mt}")
            for jc in range(NJT):
                nc.tensor.matmul(
                    op[:],
                    wo_sb[:, jc, mt * 128:(mt + 1) * 128],
                    a_t[:, jc, qc * QT:(qc + 1) * QT],
                    start=(jc == 0), stop=(jc == NJT - 1))
            ot = sp.tile([128, QT], F32, tag="ot", bufs=3,
                         name=f"ot{qc}{mt}")
            nc.vector.tensor_copy(ot[:], op[:])
            nc.sync.dma_start(
                outT[mt * 128:(mt + 1) * 128, qc * QT:(qc + 1) * QT],
                ot[:])

        def outproj_qc(qc):
            for mt in range(NMT):
                op = ps_st.tile([128, QT], F32, tag="st", name=f"op{qc}{mt}")
                for jc in range(NJT):
                    nc.tensor.matmul(
                        op[:],
                        wo_sb[:, jc, mt * 128:(mt + 1) * 128],
                        a_t[:, jc, qc * QT:(qc + 1) * QT],
                        start=(jc == 0), stop=(jc == NJT - 1))
                ot = sp.tile([128, QT], F32, tag="ot", bufs=3,
                             name=f"ot{qc}{mt}")
                nc.vector.tensor_copy(ot[:], op[:])
                nc.sync.dma_start(
                    outT[mt * 128:(mt + 1) * 128, qc * QT:(qc + 1) * QT],
                    ot[:])

        def proj_units(pair):
            preq = qw.tile([128, S], BF16, tag="preq", name=f"preq{pair}")
            prek = qw.tile([128, S], BF16, tag="prek", name=f"prek{pair}")
            for tn in range(NQT):
                yield lambda tn=tn: proj_unit(pair, "q", wq_sb, preq, tn)
            yield lambda: rope_unit(pair, "q", preq, qrot)
            for tn in range(NQT):
                yield lambda tn=tn: proj_unit(pair, "k", wk_sb, prek, tn)
            yield lambda: rope_unit(pair, "k", prek, krot)

        # per qc: how many next-pair proj units to drip in after each group
        # (placed mid-stream so the scores pipeline stays primed)
        UNIT_BUDGET = {0: 1, 1: 2, 2: 3, 3: 4}

        from collections import deque
        filler = deque()

        for u in proj_units(0):
            u()
        for pair in range(NJT):
            if pair + 1 < NJT:
                filler.extend(proj_units(pair + 1))
            for qc in range(NQT):
                ngrp = 2 * qc + 2
                ov0 = ps_ov.tile([65, QT], F32, tag="ov",
                                 name=f"ov0_{pair}{qc}")
                ov1 = ps_ov.tile([65, QT], F32, tag="ov",
                                 name=f"ov1_{pair}{qc}")
                budget = UNIT_BUDGET[qc]
                pend = None
                for g in range(ngrp):
                    sts = emit_scores(pair, qc, g)
                    if pend is not None:
                        pg, p0, p1 = pend
                        emit_tail(pair, qc, pg, p0, p1, ov0, ov1, last=False)
                    pend = (g, sts[0], sts[1])
                    if g >= 1 and budget > 0 and filler:
                        filler.popleft()()
                        budget -= 1
                pg, p0, p1 = pend
                emit_tail(pair, qc, pg, p0, p1, ov0, ov1, last=True)
                den, ous = emit_evac(pair, qc, ov0, ov1)
                rbs = emit_recip(pair, qc, den)
                emit_div(pair, qc, ous, rbs)

            while filler:
                filler.popleft()()

        for qc in range(NQT):
            outproj_qc(qc)

    nc.compile()
    return nc


_NC_CACHE = {}


def _get_nc():
    if "nc" not in _NC_CACHE:
        _NC_CACHE["nc"] = _build_nc()
    return _NC_CACHE["nc"]


def _host_prep(x, wq, wk, wv, wo, token_positions):
    head_perm = np.concatenate([np.arange(0, DK, 2), np.arange(1, DK, 2)])
    pos = np.asarray(token_positions).astype(np.float32)
    half = np.arange(0, DK, 2, dtype=np.float32) / DK
    inv_freq = THETA ** (-half)
    ang = pos[:, None] * inv_freq[None, :]        # [S, 32]
    cosT = np.cos(ang).T.astype(np.float32)       # [32, S]
    sinT = np.sin(ang).T.astype(np.float32)
    c128 = np.tile(cosT, (4, 1)).astype(bf16)     # [128, S]
    s128 = np.concatenate([-sinT, sinT, -sinT, sinT], 0).astype(bf16)

    kp = np.arange(128)[:, None, None]
    jj = np.arange(4)[None, :, None]
    qf = np.arange(QT)[None, None, :]
    maskd = (qf >= kp + 128 * jj).astype(bf16)    # [128, 4, QT]

    def prep_qk(w, g):
        rows = w.reshape(H, DK, D)[g * HG:(g + 1) * HG][:, head_perm]
        return np.ascontiguousarray(rows.reshape(HG * DK, D).T).astype(bf16)

    def prep_v(w, g):
        rows = w.reshape(H, DK, D)[g * HG:(g + 1) * HG]
        return np.ascontiguousarray(rows.reshape(HG * DK, D).T).astype(bf16)

    common = {"c128": c128, "s128": s128, "maskd": maskd}
    in_maps = []
    for c in range(NCORES):
        b, g = c // NG, c % NG
        m = dict(common)
        m["xT"] = np.ascontiguousarray(x[b].T).astype(bf16)
        m["wqT"] = prep_qk(wq, g)
        m["wkT"] = prep_qk(wk, g)
        m["wvT"] = prep_v(wv, g)
        m["woT"] = np.ascontiguousarray(wo[:, g * HG * DK:(g + 1) * HG * DK].T
                                        ).astype(bf16)
        in_maps.append(m)
    return in_maps


def kernel(x, wq, wk, wv, wo, token_positions, _trace=False):
    x = np.asarray(x, dtype=np.float32)
    in_maps = _host_prep(x, wq, wk, wv, wo, token_positions)
    nc = _get_nc()
    res = run_bass_kernel_spmd(nc, in_maps, core_ids=list(range(NCORES)),
                               trace=_trace)
    out = np.zeros((B, S, D), np.float32)
    for b in range(B):
        acc = res.results[2 * b]["outT"].astype(np.float32) + \
            res.results[2 * b + 1]["outT"].astype(np.float32)
        out[b] = acc.T
    if _trace:
        kernel.last_results = res
    return out



# revision 2
# speedup vs baseline: 1.0405x; 1.0405x over previous
"""Trainium2 Bass kernel: causal multi-head self-attention with RoPE.

Problem: B=4, S=2048, D=1024, H=16, DK=64.  out = softmax(causal(qk^T/8)) v @ wo^T
with q,k RoPE-rotated.

Sharding: 8 cores = (batch b in 0..3) x (head-group g in 0..1, 8 heads each).
Each core computes its batch's QKV for its 8 heads, causal attention, and a
partial output projection; the host sums the two group-partials per batch.

Schedule: qc-outer phases.  For each q tile (qc), attention runs for all 4
head-pairs sequentially; projection work (v chunks for qc+1, q/k tn=qc+1
column slices, output projection for earlier qc) is dripped into the PE
stream as "filler" pops, paced per score-group, so the PE never idles and
stays at its fast DVFS p-state while the Scalar engine's exp stream (the
per-group rate limiter) catches up.  Masks/normalize use DVE 4x-mode
scalar_tensor_tensor (all-SBUF bf16).
"""
import os
import sys

for _p in ("/opt/trn_rl_repo", "/root/.axon_site/_ro/trn_rl_repo"):
    if os.path.isdir(_p) and _p not in sys.path:
        sys.path.insert(0, _p)

import numpy as np
import ml_dtypes

import concourse.bass as bass
import concourse.mybir as mybir
import concourse.tile as tile
from concourse import bacc
from concourse.bass_utils import run_bass_kernel_spmd

B, S, D, H = 4, 2048, 1024, 16
DK = D // H          # 64
HG = 8               # heads per group
NG = 2               # head groups (cores per batch)
THETA = 10000.0
NCORES = 8

BF16 = mybir.dt.bfloat16
F32 = mybir.dt.float32
bf16 = ml_dtypes.bfloat16

QT = 512             # q tile width (free dim)
NQT = S // QT        # 4
NKT = S // 128       # 16 k chunks
NJT = HG * DK // 128  # 4 j-tiles (head pairs)
NDC = D // 128       # 8 d chunks
NMT = D // 128       # 8 output m tiles

MUL = mybir.AluOpType.mult
ADD = mybir.AluOpType.add


def _build_nc():
    nc = bacc.Bacc("TRN2", target_bir_lowering=False, debug=False)
    xT = nc.dram_tensor("xT", [D, S], BF16, kind="ExternalInput").ap()
    wqT = nc.dram_tensor("wqT", [D, HG * DK], BF16, kind="ExternalInput").ap()
    wkT = nc.dram_tensor("wkT", [D, HG * DK], BF16, kind="ExternalInput").ap()
    wvT = nc.dram_tensor("wvT", [D, HG * DK], BF16, kind="ExternalInput").ap()
    woT = nc.dram_tensor("woT", [HG * DK, D], BF16, kind="ExternalInput").ap()
    c128 = nc.dram_tensor("c128", [128, S], BF16, kind="ExternalInput").ap()
    s128 = nc.dram_tensor("s128", [128, S], BF16, kind="ExternalInput").ap()
    maskd = nc.dram_tensor("maskd", [128, 4, QT], BF16, kind="ExternalInput").ap()
    outT = nc.dram_tensor("outT", [D, S], F32, kind="ExternalOutput").ap()

    from collections import deque
    from contextlib import ExitStack
    with tile.TileContext(nc) as tc, ExitStack() as stk:
        pp = stk.enter_context(tc.tile_pool(name="persist", bufs=1))
        ep = stk.enter_context(tc.tile_pool(name="epool", bufs=6))
        sp = stk.enter_context(tc.tile_pool(name="smalls", bufs=2))
        qw = stk.enter_context(tc.tile_pool(name="qkvwork", bufs=6))
        ps_st = stk.enter_context(
            tc.tile_pool(name="ps_st", bufs=2, space="PSUM"))
        ps_ov = stk.enter_context(
            tc.tile_pool(name="ps_ov", bufs=2, space="PSUM"))
        ps_qkv = stk.enter_context(
            tc.tile_pool(name="ps_qkv", bufs=2, space="PSUM"))

        # ---------------- persistent tiles ----------------
        wo_sb = pp.tile([128, NJT, D], BF16)
        m_sb = pp.tile([128, 4, QT], BF16)
        qrot = pp.tile([128, NJT, S], BF16)
        krot = pp.tile([128, NJT, S], BF16)
        v_aug = pp.tile([128, NKT, HG, 66], BF16)
        a_t = pp.tile([128, NJT, S], BF16)
        xT_sb = pp.tile([128, NDC, S], BF16)
        wq_sb = pp.tile([128, NDC, HG * DK], BF16)
        wk_sb = pp.tile([128, NDC, HG * DK], BF16)
        wv_sb = pp.tile([128, NDC, HG * DK], BF16)
        c_sb = pp.tile([128, S], BF16)
        s_sb = pp.tile([128, S], BF16)

        nc.gpsimd.memset(v_aug[:, :, :, 64:65], 1.0)

        # ---------------- input DMAs (earliest-needed first) -------------
        nc.sync.dma_start(c_sb[:], c128[:])
        nc.sync.dma_start(s_sb[:], s128[:])
        nc.sync.dma_start(m_sb[:], maskd[:])
        for dc in range(NDC):
            nc.sync.dma_start(wq_sb[:, dc, :], wqT[dc * 128:(dc + 1) * 128, :])
            nc.sync.dma_start(wk_sb[:, dc, :], wkT[dc * 128:(dc + 1) * 128, :])
        # x arrives by tn-column so the tn0 projections can start early
        for tn in range(NQT):
            for dc in range(NDC):
                nc.sync.dma_start(
                    xT_sb[:, dc, tn * QT:(tn + 1) * QT],
                    xT[dc * 128:(dc + 1) * 128, tn * QT:(tn + 1) * QT])
        for dc in range(NDC):
            nc.sync.dma_start(wv_sb[:, dc, :], wvT[dc * 128:(dc + 1) * 128, :])
        for jc in range(NJT):
            nc.sync.dma_start(wo_sb[:, jc, :], woT[jc * 128:(jc + 1) * 128, :])

        # ---------------- building blocks ----------------
        def v_chunk_pops(tt):
            """Project v for s-chunk tt (all 8 heads): 4 matmul pops + copy."""
            cell = {}
            def mm(i):
                def f():
                    if "ps" not in cell:
                        cell["ps"] = ps_qkv.tile([128, QT], F32, tag="qv",
                                                 name=f"psv{tt}")
                    for dc in (2 * i, 2 * i + 1):
                        nc.tensor.matmul(
                            cell["ps"][:],
                            xT_sb[:, dc, tt * 128:(tt + 1) * 128],
                            wv_sb[:, dc, :],
                            start=(dc == 0), stop=(dc == NDC - 1))
                return f
            def cp():
                nc.vector.tensor_copy(
                    v_aug[:, tt, :, 0:64],
                    cell["ps"][:].rearrange("p (h d) -> p h d", h=HG))
            return [mm(0), mm(1), mm(2), mm(3), cp]

        def proj_pops(pair, name, w_sb, tn):
            """q/k projection for (pair, tn window): 4 mm pops + copy pop.
            Returns (pops, pre_cell) — pre_cell['pre'] is the bf16 result."""
            cell = {}
            def mm(i):
                def f():
                    if "ps" not in cell:
                        cell["ps"] = ps_qkv.tile([128, QT], F32, tag="qv",
                                                 name=f"ps{name}{pair}{tn}")
                    for dc in (2 * i, 2 * i + 1):
                        nc.tensor.matmul(
                            cell["ps"][:],
                            w_sb[:, dc, pair * 128:(pair + 1) * 128],
                            xT_sb[:, dc, tn * QT:(tn + 1) * QT],
                            start=(dc == 0), stop=(dc == NDC - 1))
                return f
            def cp():
                cell["pre"] = qw.tile([128, QT], BF16, tag="pre",
                                      name=f"pre{name}{pair}{tn}")
                nc.vector.tensor_copy(cell["pre"][:], cell["ps"][:])
            return [mm(0), mm(1), mm(2), mm(3), cp], cell

        def rope_pops(pair, name, cell, dst, tn):
            """RoPE on pre (tn window) -> dst[:, pair, window]. 4 pops."""
            w0, w1 = tn * QT, (tn + 1) * QT
            scell = {}
            def swap():
                pre = cell["pre"]
                swp = qw.tile([128, QT], BF16, tag="swp",
                              name=f"swp{name}{pair}{tn}")
                for a in range(4):
                    lo, sw = 32 * a, 32 * (a ^ 1)
                    nc.sync.dma_start(swp[lo:lo + 32, :], pre[sw:sw + 32, :])
                scell["swp"] = swp
            def mul_c():
                nc.vector.scalar_tensor_tensor(
                    dst[:, pair, w0:w1], cell["pre"][:], 1.0, c_sb[:, w0:w1],
                    op0=MUL, op1=MUL)
            def mul_s():
                swp = scell["swp"]
                nc.vector.scalar_tensor_tensor(
                    swp[:], swp[:], 1.0, s_sb[:, w0:w1], op0=MUL, op1=MUL)
            def acc():
                nc.vector.scalar_tensor_tensor(
                    dst[:, pair, w0:w1], scell["swp"][:], 1.0,
                    dst[:, pair, w0:w1], op0=MUL, op1=ADD)
            return [swap, mul_c, mul_s, acc]

        def qk_tn_pops(tn):
            """All pairs' q&k projections + rope for column window tn."""
            pops = []
            for pair in range(NJT):
                pq, cq = proj_pops(pair, "q", wq_sb, tn)
                pops += pq
                pops += rope_pops(pair, "q", cq, qrot, tn)
                pk, ck = proj_pops(pair, "k", wk_sb, tn)
                pops += pk
                pops += rope_pops(pair, "k", ck, krot, tn)
            return pops

        def outproj_pops(qc, mt):
            """One output-projection m-tile for q window qc: mm pop + store."""
            cell = {}
            def mm():
                cell["op"] = ps_qkv.tile([128, QT], F32, tag="qv",
                                         name=f"op{qc}{mt}")
                for jc in range(NJT):
                    nc.tensor.matmul(
                        cell["op"][:],
                        wo_sb[:, jc, mt * 128:(mt + 1) * 128],
                        a_t[:, jc, qc * QT:(qc + 1) * QT],
                        start=(jc == 0), stop=(jc == NJT - 1))
            def store():
                ot = sp.tile([128, QT], F32, tag="ot", bufs=3,
                             name=f"ot{qc}{mt}")
                nc.vector.tensor_copy(ot[:], cell["op"][:])
                nc.sync.dma_start(
                    outT[mt * 128:(mt + 1) * 128, qc * QT:(qc + 1) * QT],
                    ot[:])
            return [mm, store]

        # ---------------- attention building blocks ----------------
        def emit_scores(pair, qc, g):
            st0 = ps_st.tile([128, 2 * QT], F32, tag="st",
                             name=f"st0_{pair}{qc}{g}")
            st1 = ps_st.tile([128, 2 * QT], F32, tag="st",
                             name=f"st1_{pair}{qc}{g}")
            for h01, st in ((0, st0), (1, st1)):
                lo = 64 * h01
                for half in range(2):
                    kc = 2 * g + half
                    nc.tensor.matmul(
                        st[:, half * QT:(half + 1) * QT],
                        krot[lo:lo + 64, pair, kc * 128:(kc + 1) * 128],
                        qrot[lo:lo + 64, pair, qc * QT:(qc + 1) * QT],
                        start=True, stop=True,
                        tile_position=(lo, 0))
            return st0, st1

        def emit_exp(pair, qc, g, st0, st1):
            es = []
            for h01, st in ((0, st0), (1, st1)):
                e = ep.tile([128, 2 * QT], BF16, tag="e",
                            name=f"e{pair}{qc}{g}{h01}")
                nc.scalar.activation(
                    e[:], st[:], mybir.ActivationFunctionType.Exp,
                    scale=0.125)
                if g >= 2 * qc:    # diagonal band: causal mask (DVE 4x)
                    par = g - 2 * qc
                    e3 = e[:].rearrange("p (a q) -> p a q", a=2)
                    nc.vector.scalar_tensor_tensor(
                        e3, e3, 1.0, m_sb[:, 2 * par:2 * par + 2, :],
                        op0=MUL, op1=MUL)
                es.append(e)
            return es

        def emit_tail(pair, qc, g, es, ov0, ov1, last):
            for h01, ov in ((0, ov0), (1, ov1)):
                e = es[h01]
                for half in range(2):
                    kc = 2 * g + half
                    nc.tensor.matmul(
                        ov[:],
                        v_aug[:, kc, 2 * pair + h01, 0:65],
                        e[:, half * QT:(half + 1) * QT],
                        start=(kc == 0),
                        stop=(last and half == 1))

        def emit_evac(pair, qc, ov0, ov1):
            """Free the ov PSUM banks and stage the denominators."""
            den = sp.tile([2, QT], F32, tag="den", bufs=3,
                          name=f"den{pair}{qc}")
            ous = []
            for h01, ov in ((0, ov0), (1, ov1)):
                ou = ep.tile([65, QT], BF16, tag="ou", bufs=6,
                             name=f"ou{pair}{qc}{h01}")
                nc.vector.tensor_copy(ou[:], ov[:])
                nc.gpsimd.dma_start(den[h01:h01 + 1, :], ou[64:65, :])
                ous.append(ou)
            return den, ous

        def emit_recip(pair, qc, den):
            recip = sp.tile([2, QT], F32, tag="recip", name=f"rcp{pair}{qc}")
            nc.vector.reciprocal_approx_fast(recip[:], den[:])
            rbs = []
            for h01 in range(2):
                rb = sp.tile([64, QT], BF16, tag="rb", bufs=5,
                             name=f"rb{pair}{qc}{h01}")
                nc.gpsimd.dma_start(
                    rb[:],
                    recip[h01:h01 + 1, :]
                    .unsqueeze(1).to_broadcast((1, 64, QT)))
                rbs.append(rb)
            return rbs

        def emit_div(pair, qc, ous, rbs):
            nc.vector.scalar_tensor_tensor(
                a_t[0:64, pair, qc * QT:(qc + 1) * QT],
                ous[0][0:64, :], 1.0, rbs[0][:], op0=MUL, op1=MUL)
            an = sp.tile([64, QT], BF16, tag="an", bufs=3,
                         name=f"an{pair}{qc}")
            nc.vector.scalar_tensor_tensor(
                an[:], ous[1][0:64, :], 1.0, rbs[1][:], op0=MUL, op1=MUL)
            nc.sync.dma_start(
                a_t[64:128, pair, qc * QT:(qc + 1) * QT], an[:])

        # ---------------- preamble: q/k tn0 + v chunks 0-3 ----------------
        for f in qk_tn_pops(0):
            f()
        for tt in range(4):
            for f in v_chunk_pops(tt):
                f()

        # ---------------- main qc-outer attention loop ----------------
        # Deferral FIFO: [(stage, payload)...]; recip one pair-step after
        # evac, div one more step after — keeps the den-DMA/recip/broadcast
        # latency chain off the ov-ring critical path.
        defer = deque()

        def defer_step(flush=False):
            rounds = 2 if flush else 1
            while rounds > 0:
                n = len(defer)
                for _ in range(n):
                    kind, pay = defer.popleft()
                    if kind == "recip":
                        pair, qc, den, ous = pay
                        rbs = emit_recip(pair, qc, den)
                        defer.append(("div", (pair, qc, ous, rbs)))
                    else:
                        pair, qc, ous, rbs = pay
                        emit_div(pair, qc, ous, rbs)
                rounds -= 1
                if flush and not defer:
                    break

        for qc in range(NQT):
            # phase filler queue
            fillers = deque()
            if qc < 3:
                for tt in range(4 * (qc + 1), 4 * (qc + 2)):
                    fillers.extend(v_chunk_pops(tt))
                fillers.extend(qk_tn_pops(qc + 1))
            else:
                for oqc in range(3):
                    for mt in range(NMT):
                        fillers.extend(outproj_pops(oqc, mt))

            ngrp = 2 * qc + 2
            n_groups_left = ngrp * NJT
            for pair in range(NJT):
                ov0 = ps_ov.tile([65, QT], F32, tag="ov",
                                 name=f"ov0_{pair}{qc}")
                ov1 = ps_ov.tile([65, QT], F32, tag="ov",
                                 name=f"ov1_{pair}{qc}")
                pend = None
                for g in range(ngrp):
                    sts = emit_scores(pair, qc, g)
                    es = emit_exp(pair, qc, g, *sts)
                    if pend is not None:
                        emit_tail(pair, qc, pend[0], pend[1], ov0, ov1,
                                  last=False)
                    pend = (g, es)
                    # drain fillers evenly across the phase's groups
                    npop = -(-len(fillers) // n_groups_left)  # ceil
                    for _ in range(npop):
                        fillers.popleft()()
                    n_groups_left -= 1
                emit_tail(pair, qc, pend[0], pend[1], ov0, ov1, last=True)
                den, ous = emit_evac(pair, qc, ov0, ov1)
                defer.append(("recip", (pair, qc, den, ous)))
                defer_step()
            while fillers:
                fillers.popleft()()

        defer_step(flush=True)

        # ---------------- tail: output projection for qc3 ----------------
        for mt in range(NMT):
            for f in outproj_pops(3, mt):
                f()

    nc.compile()
    return nc


_NC_CACHE = {}


def _get_nc():
    if "nc" not in _NC_CACHE:
        _NC_CACHE["nc"] = _build_nc()
    return _NC_CACHE["nc"]


def _host_prep(x, wq, wk, wv, wo, token_positions):
    head_perm = np.concatenate([np.arange(0, DK, 2), np.arange(1, DK, 2)])
    pos = np.asarray(token_positions).astype(np.float32)
    half = np.arange(0, DK, 2, dtype=np.float32) / DK
    inv_freq = THETA ** (-half)
    ang = pos[:, None] * inv_freq[None, :]        # [S, 32]
    cosT = np.cos(ang).T.astype(np.float32)       # [32, S]
    sinT = np.sin(ang).T.astype(np.float32)
    c128 = np.tile(cosT, (4, 1)).astype(bf16)     # [128, S]
    s128 = np.concatenate([-sinT, sinT, -sinT, sinT], 0).astype(bf16)

    kp = np.arange(128)[:, None, None]
    jj = np.arange(4)[None, :, None]
    qf = np.arange(QT)[None, None, :]
    maskd = (qf >= kp + 128 * jj).astype(bf16)    # [128, 4, QT]

    def prep_qk(w, g):
        rows = w.reshape(H, DK, D)[g * HG:(g + 1) * HG][:, head_perm]
        return np.ascontiguousarray(rows.reshape(HG * DK, D).T).astype(bf16)

    def prep_v(w, g):
        rows = w.reshape(H, DK, D)[g * HG:(g + 1) * HG]
        return np.ascontiguousarray(rows.reshape(HG * DK, D).T).astype(bf16)

    common = {"c128": c128, "s128": s128, "maskd": maskd}
    in_maps = []
    for c in range(NCORES):
        b, g = c // NG, c % NG
        m = dict(common)
        m["xT"] = np.ascontiguousarray(x[b].T).astype(bf16)
        m["wqT"] = prep_qk(wq, g)
        m["wkT"] = prep_qk(wk, g)
        m["wvT"] = prep_v(wv, g)
        m["woT"] = np.ascontiguousarray(wo[:, g * HG * DK:(g + 1) * HG * DK].T
                                        ).astype(bf16)
        in_maps.append(m)
    return in_maps


def kernel(x, wq, wk, wv, wo, token_positions, _trace=False):
    x = np.asarray(x, dtype=np.float32)
    in_maps = _host_prep(x, wq, wk, wv, wo, token_positions)
    nc = _get_nc()
    res = run_bass_kernel_spmd(nc, in_maps, core_ids=list(range(NCORES)),
                               trace=_trace)
    out = np.zeros((B, S, D), np.float32)
    for b in range(B):
        acc = res.results[2 * b]["outT"].astype(np.float32) + \
            res.results[2 * b + 1]["outT"].astype(np.float32)
        out[b] = acc.T
    if _trace:
        kernel.last_results = res
    return out
